# revision 45
# baseline (speedup 1.0000x reference)
"""Causal multi-head self-attention (B=8, S=2048, D=384, H=4, Hd=96) on 8
Trainium2 NeuronCores — v3.

Sharding: data-parallel over batch — each core processes one batch element,
weights replicated. No collectives.

v3 changes vs v2 (exp split across engines, DVE offload, DMA consolidation):
  - softmax exp is split between the Activation engine (true exp -> fp8) and
    the Vector engine: for 4 of the 12 fp8 diagonal (chunk, head) groups, DVE
    computes fp8 *bit patterns* directly as round(1.17796*qk + 32) into a
    uint8 tile (the int value IS the biased exponent+mantissa of fp8e4m3, a
    piecewise-linear exp of scale*qk - 3*ln2), which the PE reads via bitcast
    as fp8e4m3.  HW f32->u8 saturation clamps the negative tail to 0 exactly
    like fp8 denormal truncation.  ACT pairs use bias=-3*ln2 so both engines
    produce the same per-row scale (cancels in normalization).  The DVE op
    fuses the causal mask: scalar_tensor_tensor adds a mask tile that is +32
    valid / -448 masked (saturates the u8 cast to 0).  ATTN_AD tunes the
    ACT/DVE diagonal split (default 8 on ACT, 4 on DVE).
  - V' drain writes only the 96 real dims per head block (strided AP); the
    ones-column (softmax denominator) is memset once.  A fp8-DoubleRow V'
    projection exists behind ATTN_V=vdr (faster, but doubles output l2 err).
  - output-projection drain moved to the ACT engine as a pure Copy; bo is
    folded into row 0 of each Wo head block (on[0] == 1.0 after normalize).
    Outputs accumulate in a [128,4,384] staging tile, one DMA per chunk.
  - DMAs consolidated by host-side packing (wq/wk/wv/wo/masks one DMA each):
    HWDGE descriptor generation (~625ns per DMA, serialized per queue) made
    many small DMAs the dominant startup cost.
  - gpsimd (Pool) is kept OFF the exp->PV critical path (its tensor ops are
    several x slower on HW than the cost model says); it only runs the
    partition_broadcast of 1/denominator, which sits off the critical chain.
"""

import sys

sys.path.insert(0, "/opt/trn_rl_repo")

import numpy as np
import ml_dtypes

import concourse.bass as bass
import concourse.tile as tile
from concourse import bacc, mybir
from concourse.bass_utils import run_bass_kernel_spmd

N_CORES = 8
S = 2048
D = 384
H = 4
HD = 96
CH = 512          # q-chunk width (columns per matmul)
NCH = S // CH     # 4 q-chunks
P = 128           # k-tile height / partition dim
KTN = S // P      # 16 k-tiles
SCALE = 1.0 / np.sqrt(HD)

# u8-exp: fp8e4m3 bits y = round(A_U8 * qk_raw + 32); value(y) = exp(z - 3ln2)
# for z = SCALE*qk_raw.  ACT pairs use exp(scale*qk + B_FP8) to match.
L2E8 = 8.0 / np.log(2.0)
A_U8 = float(np.float32(L2E8 * SCALE))
ADD32 = 32.0
B_FP8 = float(np.float32(-3.0 * np.log(2.0)))

F32 = mybir.dt.float32
BF16 = mybir.dt.bfloat16
F8 = mybir.dt.float8e4
U8 = mybir.dt.uint8
DR = mybir.MatmulPerfMode.DoubleRow

import os
VARIANT = set(v for v in os.environ.get("ATTN_V", "").split(",") if v)

# diagonal (ci,h) groups exp'd on ACT (+mask op) instead of DVE (fused mask).
# ATTN_AD = how many of the 12 fp8 diag groups go to ACT (0..12).
_AD_ORDER = [(1, 2), (1, 3), (2, 2), (2, 3), (3, 2), (3, 3),
             (1, 0), (1, 1), (2, 0), (2, 1), (3, 0), (3, 1)]
_n_ad = 12 if "nou8" in VARIANT else int(os.environ.get("ATTN_AD", "8"))
ACT_DIAG = set(_AD_ORDER[:_n_ad])
# off-diagonal (ci,h,pr) pairs moved from ACT exp to DVE u8 exp
_n_od = int(os.environ.get("ATTN_OD", "0"))
_OD_ORDER = [(ci, h, pr) for ci in range(3, 0, -1) for pr in range(2 * ci)
             for h in range(H)]
DVE_OFF = set(_OD_ORDER[:_n_od])


def build_nc(repeat=1, variant=(), loop_n=0):
    nc = bacc.Bacc("TRN2", target_bir_lowering=False, debug=False,
                   enable_asserts=False, num_devices=N_CORES)

    xt_d = nc.dram_tensor("xt", [D, S], BF16, kind="ExternalInput").ap()
    x8p_d = nc.dram_tensor("x8p", [P, 3, S], F8, kind="ExternalInput").ap()
    wqp_d = nc.dram_tensor("wqp", [P, 3 * D], BF16, kind="ExternalInput").ap()
    wkp_d = nc.dram_tensor("wkp", [P, 3 * D], BF16, kind="ExternalInput").ap()
    wv8p_d = nc.dram_tensor("wv8p", [P, 3, D], F8, kind="ExternalInput").ap()
    wvp_d = nc.dram_tensor("wvp", [P, 3 * D], BF16, kind="ExternalInput").ap()
    wop_d = nc.dram_tensor("wop", [97, H * D], BF16, kind="ExternalInput").ap()
    bqk_d = nc.dram_tensor("bqk", [HD, 2 * H], F32, kind="ExternalInput").ap()
    vbb_d = nc.dram_tensor("vbb", [P, D], F32, kind="ExternalInput").ap()
    msk_d = nc.dram_tensor("msk", [P, P], BF16, kind="ExternalInput").ap()
    m8_d = nc.dram_tensor("m8", [P, 2, 768], F8, kind="ExternalInput").ap()
    out_d = nc.dram_tensor("out", [S, D], F32, kind="ExternalOutput").ap()

    Exp = mybir.ActivationFunctionType.Exp
    mult = mybir.AluOpType.mult
    add = mybir.AluOpType.add

    with tile.TileContext(nc) as tc:
        wpool = tc.alloc_tile_pool(name="w", bufs=1)
        xpool = tc.alloc_tile_pool(name="x", bufs=1)
        qkt_pool = tc.alloc_tile_pool(name="qkt", bufs=1)
        vpool = tc.alloc_tile_pool(name="v", bufs=1)
        ppool = tc.alloc_tile_pool(name="p", bufs=12)
        onpool = tc.alloc_tile_pool(name="on", bufs=2)
        rpool = tc.alloc_tile_pool(name="r", bufs=4)
        qkpool = tc.alloc_tile_pool(name="qkps", bufs=2, space="PSUM")
        accpool = tc.alloc_tile_pool(name="accps", bufs=2, space="PSUM")
        fpool = tc.alloc_tile_pool(name="fps", bufs=2, space="PSUM")

        import contextlib
        loop_ctx = (tc.For_i(0, loop_n, 1) if loop_n
                    else contextlib.nullcontext())
        with loop_ctx:
          for _rep in range(repeat):
            # ---- loads: bf16 weights + x on SP queue, fp8 x pack on ACT ----
            # interleave so the first Q-proj matmul's operands (wq, xt0)
            # arrive first
            wqp = wpool.tile([P, 3 * D], BF16, name="wqp", tag="wqp")
            nc.sync.dma_start(wqp[:], wqp_d[:, :])
            xt_sb = [xpool.tile([P, S], BF16, name=f"xt{t}", tag=f"xt{t}")
                     for t in range(3)]
            nc.sync.dma_start(xt_sb[0][:], xt_d[0:P, :])
            wkp = wpool.tile([P, 3 * D], BF16, name="wkp", tag="wkp")
            nc.sync.dma_start(wkp[:], wkp_d[:, :])
            nc.sync.dma_start(xt_sb[1][:], xt_d[P:2 * P, :])
            nc.sync.dma_start(xt_sb[2][:], xt_d[2 * P:3 * P, :])
            if "vdr" in VARIANT:
                x8p = xpool.tile([P, 3, S], F8, name="x8p", tag="x8p")
                nc.scalar.dma_start(x8p[:, :, :], x8p_d[:, :, :])
                wv8p = wpool.tile([P, 3, D], F8, name="wv8p", tag="wv8p")
                nc.sync.dma_start(wv8p[:, :, :], wv8p_d[:, :, :])
            # secondary constants ride the ACT DGE queue: HWDGE descriptor
            # generation (~625ns/DMA) is serialized per queue, and these
            # would otherwise delay the first x/weight arrivals on SP
            bqk_sb = wpool.tile([HD, 2 * H], F32, name="bqk", tag="bqk")
            nc.sync.dma_start(bqk_sb[:], bqk_d[:, :])
            wvp = wpool.tile([P, 3 * D], BF16, name="wvp", tag="wvp")
            nc.sync.dma_start(wvp[:], wvp_d[:, :])
            wvb_sb = [wvp[:, D * t:D * t + D] for t in range(3)]
            vbb_sb = wpool.tile([P, D], F32, name="vbb", tag="vbb")
            nc.sync.dma_start(vbb_sb[:], vbb_d[:, :])
            msk_sb = wpool.tile([P, P], BF16, name="msk", tag="msk")
            nc.sync.dma_start(msk_sb[:], msk_d[:, :])
            m8_sb = wpool.tile([P, 2, 768], F8, name="m8", tag="m8")
            nc.sync.dma_start(m8_sb[:, :, :], m8_d[:, :, :])
            wop = wpool.tile([97, H * D], BF16, name="wop", tag="wop")
            nc.sync.dma_start(wop[:], wop_d[:, :])
            sh8_sb = wpool.tile([P, 1], F32, name="sh8", tag="sh8")
            nc.vector.memset(sh8_sb[:], B_FP8)
            one97 = wpool.tile([1, 97], F32, name="one97", tag="one97")
            nc.vector.memset(one97[:], 1.0)
            # views: composite Pool mask [.,.,0:256], DVE additive mask rest
            wq_sb = [wqp[:, D * t:D * t + D] for t in range(3)]
            wk_sb = [wkp[:, D * t:D * t + D] for t in range(3)]
            wo_sb = [wop[:, D * h:D * h + D] for h in range(H)]

            qt_sb, kt_sb = [], []
            for h in range(H):
                qt = qkt_pool.tile([HD, S], BF16, name=f"qt{h}", tag=f"qt{h}")
                qt_sb.append(qt)
                kt = qkt_pool.tile([HD, S], BF16, name=f"kt{h}", tag=f"kt{h}")
                kt_sb.append(kt)

            # V tiles: per head 128-col block, col 0 = ones (denominator),
            # cols 1..96 = dims, 97..127 junk pads (land in unread acc rows)
            vall8 = vpool.tile([P, KTN, 4 * P], F8, name="vall8", tag="vall8")
            ones8 = bass.AP(vall8.tensor, vall8[:, 0, 0:1].offset,
                            [list(vall8[:, 0, 0:1].ap[0]), [4 * P, KTN], [P, H]])
            nc.vector.memset(ones8, 1.0)
            vallb = vpool.tile([P, 4, 97 * H], BF16, name="vallb", tag="vallb")
            onesb = bass.AP(vallb.tensor, vallb[:, 0, 0:1].offset,
                            [list(vallb[:, 0, 0:1].ap[0]), [97 * H, 4], [97, H]])
            nc.vector.memset(onesb, 1.0)

            def proj_units(ci):
                """Projection work units for chunk ci: 8 Q/K + 4 V' closures."""
                units = []

                def qk_unit(w_sb, boff, dst, h):
                    def emit():
                        ps = fpool.tile([HD, CH], F32, name="projps", tag="f")
                        for t in range(3):
                            nc.tensor.matmul(
                                ps[:],
                                w_sb[t][:, HD * h:HD * h + HD],
                                xt_sb[t][:, CH * ci:CH * ci + CH],
                                start=(t == 0), stop=(t == 2))
                        nc.vector.tensor_scalar_add(
                            dst[h][:, CH * ci:CH * ci + CH], ps[:],
                            bqk_sb[:, boff + h:boff + h + 1])
                    return emit

                def v_unit(st):
                    def emit():
                        vbs = bass.AP(vbb_sb.tensor, vbb_sb.offset,
                                      [list(vbb_sb[:, 0:1].ap[0]), [HD, H], [1, HD]])
                        ps = fpool.tile([P, D], F32, name="vps", tag="f")
                        if "vdr" in VARIANT:
                            # fp8 DoubleRow projection: faster, but the fp8
                            # x/Wv noise roughly doubles the output l2 error
                            nc.tensor.matmul(ps[:],
                                             x8p[:, 0:2, P * st:P * st + P],
                                             wv8p[:, 0:2, :], start=True, stop=False,
                                             perf_mode=DR, skip_group_check=True)
                            nc.tensor.matmul(ps[:],
                                             x8p[:, 2, P * st:P * st + P],
                                             wv8p[:, 2, :], start=False, stop=True,
                                             skip_group_check=True)
                        else:
                            for t in range(3):
                                nc.tensor.matmul(ps[:],
                                                 xt_sb[t][:, P * st:P * st + P],
                                                 wvb_sb[t][:],
                                                 start=(t == 0), stop=(t == 2))
                        src = bass.AP(ps.tensor, ps.offset,
                                      [list(ps[:, 0:1].ap[0]), [HD, H], [1, HD]])
                        vdst = vall8[:, st, 0:1]
                        dst = bass.AP(vdst.tensor, vdst.offset + 1,
                                      [list(vdst.ap[0]), [P, H], [1, HD]])
                        nc.vector.tensor_tensor(dst, src, vbs, op=add)
                        if st < 4:
                            bdst0 = vallb[:, st, 0:1]
                            bdst = bass.AP(bdst0.tensor, bdst0.offset + 1,
                                           [list(bdst0.ap[0]), [97, H], [1, HD]])
                            nc.vector.tensor_tensor(bdst, src, vbs, op=add)
                    return emit

                for h in range(H):
                    units.append(qk_unit(wq_sb, 0, qt_sb, h))
                    units.append(qk_unit(wk_sb, H, kt_sb, h))
                for st in range(4 * ci, 4 * ci + 4):
                    units.append(v_unit(st))
                return units

            def outproj_units(ci, on_tiles, last=False):
                fs = onpool.tile([P, 4, D], F32, name="fs", tag="fs", bufs=2)
                units = []

                def o_unit(sj):
                    def emit():
                        fo = fpool.tile([P, D], F32, name="fo", tag="f")
                        for h in range(H):
                            nc.tensor.matmul(fo[:], on_tiles[h][:, P * sj:P * sj + P],
                                             wo_sb[h][:], start=(h == 0), stop=(h == 3))
                        if "outdve" in VARIANT:
                            nc.vector.tensor_copy(fs[:, sj, :], fo[:])
                        else:
                            nc.scalar.copy(fs[:, sj, :], fo[:])
                    return emit

                def dma_unit():
                    dst = bass.AP(out_d.tensor, 512 * ci * D,
                                  [[D, P], [P * D, 4], [1, D]])
                    nc.sync.dma_start(dst, fs[:, :, :])

                def sj_dma_unit(sj):
                    def emit():
                        dst = bass.AP(out_d.tensor, (512 * ci + P * sj) * D,
                                      [[D, P], [1, D]])
                        nc.sync.dma_start(dst, fs[:, sj, :])
                    return emit

                if last:
                    # final chunk: per-tile DMAs so the store overlaps the
                    # remaining output-projection compute in the tail
                    for sj in range(4):
                        units.append(o_unit(sj))
                        units.append(sj_dma_unit(sj))
                else:
                    for sj in range(4):
                        units.append(o_unit(sj))
                    units.append(dma_unit)
                return units

            def attend_chunk(ci, filler):
                """Attention for chunk ci; between pair iterations, emit
                independent filler units to keep PE fed during exp waits."""
                on_tiles = []
                nkt = 4 * (ci + 1)
                npairs = H * (nkt // 2)
                fill_every = max(1, npairs // max(1, len(filler)))
                fi = 0
                pair_no = 0
                fp8 = ci >= 1

                def emit_pair_a(h, pr):
                    """QK matmuls + exp/mask for one pair; returns the PV
                    closure so both heads' exps are issued before either
                    PV — PE executes in order, so this lets head h's exp
                    overlap head h+1's QK instead of stalling on PV(h)."""
                    qk = qkpool.tile([P, 2, CH], F32, name="qk", tag="qk")
                    kt0 = 2 * pr
                    rt0 = P * kt0 - CH * ci
                    sc = max(rt0, 0)
                    diag = rt0 >= 0
                    for j in range(2):
                        nc.tensor.matmul(
                            qk[:, j, sc:CH],
                            kt_sb[h][:, P * (kt0 + j):P * (kt0 + j) + P],
                            qt_sb[h][:, CH * ci + sc:CH * (ci + 1)],
                            start=True, stop=True)
                    if not fp8:
                        # chunk 0: bf16 path, ACT exp + DVE strided mask
                        pt = ppool.tile([P, 2, CH], BF16, name="pt", tag="pt")
                        nc.scalar.activation(pt[:, :, sc:CH], qk[:, :, sc:CH],
                                             Exp, scale=float(SCALE))
                        if diag:
                            base = pt[:, 0, 0:1]
                            diag_view = bass.AP(
                                base.tensor, base.offset + rt0,
                                [[2 * CH, P], [CH + P, 2], [1, P]])
                            mbc = msk_sb[:].unsqueeze(1).broadcast_to([P, 2, P])
                            nc.vector.tensor_tensor(diag_view, diag_view,
                                                    mbc, op=mult)

                        def do_pv(acc):
                            for j in range(2):
                                kt = kt0 + j
                                scol = max(P * kt - CH * ci, 0)
                                nc.tensor.matmul(
                                    acc[0:97, scol:CH],
                                    vallb[:, kt, 97 * h:97 * h + 97],
                                    pt[:, j, scol:CH],
                                    start=(kt == 0), stop=(kt == nkt - 1),
                                    skip_group_check=True)
                        return do_pv

                    use_dve = ((diag and (ci, h) not in ACT_DIAG)
                               or (not diag and (ci, h, pr) in DVE_OFF))
                    if use_dve:
                        # u8 piecewise exp, causal mask fused additively
                        pt = ppool.tile([P, 2, CH], U8, name="pt", tag="pt")
                        if diag:
                            nc.vector.scalar_tensor_tensor(
                                pt[:, :, sc:CH], qk[:, :, sc:CH], A_U8,
                                m8_sb[:, :, 256:256 + CH - sc],
                                op0=mult, op1=add)
                        else:
                            nc.vector.tensor_scalar(
                                pt[:, :, :], qk[:, :, :], A_U8, ADD32,
                                op0=mult, op1=add)
                        pt_mv = pt[:, :, sc:CH].bitcast(F8)
                    else:
                        pt = ppool.tile([P, 2, CH], F8, name="pt", tag="pt")
                        nc.scalar.activation(pt[:, :, sc:CH], qk[:, :, sc:CH],
                                             Exp, scale=float(SCALE),
                                             bias=sh8_sb[:])
                        if diag:
                            mask_eng = (nc.gpsimd if "poolmask" in VARIANT
                                        else nc.vector)
                            mask_eng.tensor_tensor(
                                pt[:, :, rt0:rt0 + 256],
                                pt[:, :, rt0:rt0 + 256],
                                m8_sb[:, :, 0:256], op=mult)
                        pt_mv = pt[:, :, sc:CH]

                    def do_pv(acc):
                        nc.tensor.matmul(
                            acc[:, sc:CH],
                            vall8[:, kt0:kt0 + 2, P * h:P * h + P],
                            pt_mv,
                            start=(pr == 0), stop=(pr == nkt // 2 - 1),
                            skip_group_check=True,
                            perf_mode=DR)
                    return do_pv

                for hh in range(0, H, 2):
                    acc0 = accpool.tile([P, CH], F32, name="acc", tag="acc")
                    acc1 = accpool.tile([P, CH], F32, name="acc", tag="acc")
                    for pr in range(nkt // 2):
                        pv0 = emit_pair_a(hh, pr)
                        pv1 = emit_pair_a(hh + 1, pr)
                        pv0(acc0)
                        pv1(acc1)
                        pair_no += 2
                        if pair_no % fill_every < 2 and fi < len(filler):
                            filler[fi]()
                            fi += 1
                    rec2 = rpool.tile([1, 2 * CH], F32, name="rec", tag="rec")
                    nc.vector.reciprocal_approx_fast(
                        out=rec2[0:1, 0:CH], in_=acc0[0:1, :])
                    nc.vector.reciprocal_approx_fast(
                        out=rec2[0:1, CH:2 * CH], in_=acc1[0:1, :])
                    if "pebc" in VARIANT:
                        # broadcast 1/denom across partitions on the PE:
                        # ones[1,97]^T @ rec2[1,:] -> PSUM [97, CH]
                        for kk, a in ((0, acc0), (1, acc1)):
                            rbp = fpool.tile([97, CH], F32, name="rbp", tag="f")
                            nc.tensor.matmul(
                                rbp[:], one97[:],
                                rec2[0:1, CH * kk:CH * kk + CH],
                                start=True, stop=True)
                            on = onpool.tile([97, CH], BF16, name=f"on{hh + kk}",
                                             tag=f"on{hh + kk}")
                            nc.vector.tensor_tensor(
                                on[:], a[0:97, :], rbp[:], op=mult)
                            on_tiles.append(on)
                    else:
                        rb2 = rpool.tile([97, 2 * CH], F32, name="rb", tag="rb")
                        if "fakebc" in VARIANT:
                            # timing ablation: DVE memset instead of Pool
                            # broadcast (WRONG results)
                            nc.vector.memset(rb2[:], 0.01)
                        else:
                            nc.gpsimd.partition_broadcast(rb2[:], rec2[:],
                                                          channels=97)
                        for kk, a in ((0, acc0), (1, acc1)):
                            on = onpool.tile([97, CH], BF16, name=f"on{hh + kk}",
                                             tag=f"on{hh + kk}")
                            nc.vector.tensor_tensor(
                                on[:], a[0:97, :], rb2[:, CH * kk:CH * kk + CH],
                                op=mult)
                            on_tiles.append(on)
                while fi < len(filler):
                    filler[fi]()
                    fi += 1
                return on_tiles

            # software pipeline across chunks: during attention of chunk ci,
            # emit chunk ci+1's projections and chunk ci-1's output
            # projection.  NOTE the chunk order is forced: attend(ci) needs
            # the K/V prefix from proj(0..ci), so attention must run 0,1,2,3.
            # Chunk 3's own K/V units are only consumed from pair 6 on
            # (k-tiles 12-15), so 6 of them are deferred into chunk 3's own
            # attention window, where filler is otherwise scarce.
            for u in proj_units(0):
                u()
            pending_out = []
            deferred = []
            for ci in range(NCH):
                filler = deferred + list(pending_out)
                deferred = []
                if ci + 1 < NCH:
                    filler += proj_units(ci + 1)
                on_tiles = attend_chunk(ci, filler)
                pending_out = outproj_units(ci, on_tiles, last=(ci == NCH - 1))
            for u in pending_out:
                u()

        for pool in (fpool, accpool, qkpool, rpool, onpool, ppool, vpool,
                     qkt_pool, xpool, wpool):
            pool.release()

    nc.finalize()
    return nc


_NC_CACHE = None


def get_nc():
    global _NC_CACHE
    if _NC_CACHE is None:
        _NC_CACHE = build_nc()
    return _NC_CACHE


def host_prep(x, Wq, bq, Wk, bk, Wv, bv, Wo, bo):
    """Build per-core input maps (layout prep only; all FLOPs run on device)."""
    BF = ml_dtypes.bfloat16
    F8H = ml_dtypes.float8_e4m3
    x = np.ascontiguousarray(np.asarray(x, dtype=np.float32))
    Wq = np.ascontiguousarray(np.asarray(Wq, dtype=np.float32))
    Wk = np.ascontiguousarray(np.asarray(Wk, dtype=np.float32))
    Wv = np.ascontiguousarray(np.asarray(Wv, dtype=np.float32))
    Wo = np.ascontiguousarray(np.asarray(Wo, dtype=np.float32))
    bq = np.asarray(bq, dtype=np.float32)
    bk = np.asarray(bk, dtype=np.float32)
    bv = np.asarray(bv, dtype=np.float32)
    bo = np.asarray(bo, dtype=np.float32)

    wqp = np.hstack([Wq[0:P], Wq[P:2 * P], Wq[2 * P:3 * P]]).astype(BF)
    wkp = np.hstack([Wk[0:P], Wk[P:2 * P], Wk[2 * P:3 * P]]).astype(BF)
    wvp = np.hstack([Wv[0:P], Wv[P:2 * P], Wv[2 * P:3 * P]]).astype(BF)
    wv8p = np.stack([Wv[0:P], Wv[P:2 * P], Wv[2 * P:3 * P]], axis=1).astype(F8H)
    vbb = np.ascontiguousarray(np.broadcast_to(bv.reshape(1, D), (P, D)))

    # Wo with per-head 97-row blocks; row 0 picks up bo via on[0] == 1.0
    wop = np.zeros((97, H * D), np.float32)
    for h in range(H):
        wop[0, D * h:D * h + D] = bo / H
        wop[1:97, D * h:D * h + D] = Wo[HD * h:HD * h + HD, :]

    jj = np.arange(P)[None, :]
    pp = np.arange(P)[:, None]
    msk = (jj >= pp).astype(BF)
    tri = (jj >= pp).astype(np.float32)
    # m8[:, :, 0:256]: Pool composite {0,1}: [tri|ones] slab0, [zero|tri] slab1
    # m8[:, :, 256:768]: DVE additive mask: +32 valid, -448 masked
    m8 = np.zeros((P, 2, 768), np.float32)
    m8[:, 0, 0:P] = tri
    m8[:, 0, P:256] = 1.0
    m8[:, 1, P:256] = tri
    ja = np.arange(CH)[None, :]
    m8[:, 0, 256:768] = np.where(ja >= pp, ADD32, -448.0)
    m8[:, 1, 256:768] = np.where(ja >= P + pp, ADD32, -448.0)
    m8 = m8.astype(F8H)

    bqk = np.hstack([bq.reshape(H, HD).T, bk.reshape(H, HD).T])
    bqk = np.ascontiguousarray(bqk)

    common = dict(wqp=wqp, wkp=wkp, wvp=wvp, wv8p=wv8p, wop=wop.astype(BF),
                  bqk=bqk, vbb=vbb, msk=msk, m8=m8)
    maps = []
    for b in range(x.shape[0]):
        xt = np.ascontiguousarray(x[b].T)  # [D, S]
        x8p = np.stack([xt[0:P], xt[P:2 * P], xt[2 * P:3 * P]], axis=1).astype(F8H)
        maps.append(dict(xt=xt.astype(BF), x8p=x8p, **common))
    return maps


def kernel(**inputs):
    in_maps = host_prep(**inputs)
    nc = get_nc()
    res = run_bass_kernel_spmd(nc, in_maps, core_ids=list(range(N_CORES)))
    return np.stack([res.results[b]["out"] for b in range(N_CORES)], axis=0)


# revision 57
# speedup vs baseline: 1.0978x; 1.0978x over previous
"""Causal multi-head self-attention (B=8, S=2048, D=384, H=4, Hd=96) on 8
Trainium2 NeuronCores — v3.

Sharding: data-parallel over batch — each core processes one batch element,
weights replicated. No collectives.

v3 changes vs v2 (exp split across engines, DVE offload, DMA consolidation):
  - softmax exp is split between the Activation engine (true exp -> fp8) and
    the Vector engine: for 4 of the 12 fp8 diagonal (chunk, head) groups, DVE
    computes fp8 *bit patterns* directly as round(1.17796*qk + 32) into a
    uint8 tile (the int value IS the biased exponent+mantissa of fp8e4m3, a
    piecewise-linear exp of scale*qk - 3*ln2), which the PE reads via bitcast
    as fp8e4m3.  HW f32->u8 saturation clamps the negative tail to 0 exactly
    like fp8 denormal truncation.  ACT pairs use bias=-3*ln2 so both engines
    produce the same per-row scale (cancels in normalization).  The DVE op
    fuses the causal mask: scalar_tensor_tensor adds a mask tile that is +32
    valid / -448 masked (saturates the u8 cast to 0).  ATTN_AD tunes the
    ACT/DVE diagonal split (default 8 on ACT, 4 on DVE).
  - V' drain writes only the 96 real dims per head block (strided AP); the
    ones-column (softmax denominator) is memset once.  A fp8-DoubleRow V'
    projection exists behind ATTN_V=vdr (faster, but doubles output l2 err).
  - output-projection drain moved to the ACT engine as a pure Copy; bo is
    folded into row 0 of each Wo head block (on[0] == 1.0 after normalize).
    Outputs accumulate in a [128,4,384] staging tile, one DMA per chunk.
  - DMAs consolidated by host-side packing (wq/wk/wv/wo/masks one DMA each):
    HWDGE descriptor generation (~625ns per DMA, serialized per queue) made
    many small DMAs the dominant startup cost.
  - gpsimd (Pool) is kept OFF the exp->PV critical path (its tensor ops are
    several x slower on HW than the cost model says); it only runs the
    partition_broadcast of 1/denominator, which sits off the critical chain.
"""

import sys

sys.path.insert(0, "/opt/trn_rl_repo")

import numpy as np
import ml_dtypes

import concourse.bass as bass
import concourse.tile as tile
from concourse import bacc, mybir
from concourse.bass_utils import run_bass_kernel_spmd

N_CORES = 8
S = 2048
D = 384
H = 4
HD = 96
CH = 512          # q-chunk width (columns per matmul)
NCH = S // CH     # 4 q-chunks
P = 128           # k-tile height / partition dim
KTN = S // P      # 16 k-tiles
SCALE = 1.0 / np.sqrt(HD)

# u8-exp: fp8e4m3 bits y = round(A_U8 * qk_raw + 32); value(y) = exp(z - 3ln2)
# for z = SCALE*qk_raw.  ACT pairs use exp(scale*qk + B_FP8) to match.
L2E8 = 8.0 / np.log(2.0)
A_U8 = float(np.float32(L2E8 * SCALE))
ADD32 = 32.0
B_FP8 = float(np.float32(-3.0 * np.log(2.0)))

F32 = mybir.dt.float32
BF16 = mybir.dt.bfloat16
F8 = mybir.dt.float8e4
U8 = mybir.dt.uint8
DR = mybir.MatmulPerfMode.DoubleRow

import os
VARIANT = set(v for v in os.environ.get("ATTN_V", "").split(",") if v)

# diagonal (ci,h) groups exp'd on ACT (+mask op) instead of DVE (fused mask).
# ATTN_AD = how many of the 12 fp8 diag groups go to ACT (0..12).
_AD_ORDER = [(1, 2), (1, 3), (2, 2), (2, 3), (3, 2), (3, 3),
             (1, 0), (1, 1), (2, 0), (2, 1), (3, 0), (3, 1)]
_n_ad = 12 if "nou8" in VARIANT else int(os.environ.get("ATTN_AD", "8"))
ACT_DIAG = set(_AD_ORDER[:_n_ad])
# off-diagonal (ci,h,pr) pairs moved from ACT exp to DVE u8 exp
_n_od = int(os.environ.get("ATTN_OD", "0"))
_OD_ORDER = [(ci, h, pr) for ci in range(3, 0, -1) for pr in range(2 * ci)
             for h in range(H)]
DVE_OFF = set(_OD_ORDER[:_n_od])


def build_nc(repeat=1, variant=(), loop_n=0):
    nc = bacc.Bacc("TRN2", target_bir_lowering=False, debug=False,
                   enable_asserts=False, num_devices=N_CORES)

    xt_d = nc.dram_tensor("xt", [D, S], BF16, kind="ExternalInput").ap()
    x8p_d = nc.dram_tensor("x8p", [P, 3, S], F8, kind="ExternalInput").ap()
    wqp_d = nc.dram_tensor("wqp", [P, 3 * D], BF16, kind="ExternalInput").ap()
    wkp_d = nc.dram_tensor("wkp", [P, 3 * D], BF16, kind="ExternalInput").ap()
    wv8p_d = nc.dram_tensor("wv8p", [P, 3, D], F8, kind="ExternalInput").ap()
    wvp_d = nc.dram_tensor("wvp", [P, 3 * D], BF16, kind="ExternalInput").ap()
    wop_d = nc.dram_tensor("wop", [97, H * D], BF16, kind="ExternalInput").ap()
    bqk_d = nc.dram_tensor("bqk", [HD, 2 * H], F32, kind="ExternalInput").ap()
    vbb_d = nc.dram_tensor("vbb", [P, D], F32, kind="ExternalInput").ap()
    msk_d = nc.dram_tensor("msk", [P, P], BF16, kind="ExternalInput").ap()
    m8_d = nc.dram_tensor("m8", [P, 2, 768], F8, kind="ExternalInput").ap()
    out_d = nc.dram_tensor("out", [S, D], F32, kind="ExternalOutput").ap()

    Exp = mybir.ActivationFunctionType.Exp
    mult = mybir.AluOpType.mult
    add = mybir.AluOpType.add

    with tile.TileContext(nc) as tc:
        wpool = tc.alloc_tile_pool(name="w", bufs=1)
        xpool = tc.alloc_tile_pool(name="x", bufs=1)
        qkt_pool = tc.alloc_tile_pool(name="qkt", bufs=1)
        vpool = tc.alloc_tile_pool(name="v", bufs=1)
        ppool = tc.alloc_tile_pool(name="p", bufs=12)
        onpool = tc.alloc_tile_pool(name="on", bufs=2)
        rpool = tc.alloc_tile_pool(name="r", bufs=4)
        qkpool = tc.alloc_tile_pool(name="qkps", bufs=2, space="PSUM")
        accpool = tc.alloc_tile_pool(name="accps", bufs=2, space="PSUM")
        fpool = tc.alloc_tile_pool(name="fps", bufs=2, space="PSUM")

        import contextlib
        loop_ctx = (tc.For_i(0, loop_n, 1) if loop_n
                    else contextlib.nullcontext())
        with loop_ctx:
          for _rep in range(repeat):
            # ---- loads: bf16 weights + x on SP queue, fp8 x pack on ACT ----
            # interleave so the first Q-proj matmul's operands (wq, xt0)
            # arrive first
            wqp = wpool.tile([P, 3 * D], BF16, name="wqp", tag="wqp")
            nc.sync.dma_start(wqp[:], wqp_d[:, :])
            xt_sb = [xpool.tile([P, S], BF16, name=f"xt{t}", tag=f"xt{t}")
                     for t in range(3)]
            nc.sync.dma_start(xt_sb[0][:], xt_d[0:P, :])
            wkp = wpool.tile([P, 3 * D], BF16, name="wkp", tag="wkp")
            nc.sync.dma_start(wkp[:], wkp_d[:, :])
            nc.sync.dma_start(xt_sb[1][:], xt_d[P:2 * P, :])
            nc.sync.dma_start(xt_sb[2][:], xt_d[2 * P:3 * P, :])
            if "vdr" in VARIANT:
                x8p = xpool.tile([P, 3, S], F8, name="x8p", tag="x8p")
                nc.scalar.dma_start(x8p[:, :, :], x8p_d[:, :, :])
                wv8p = wpool.tile([P, 3, D], F8, name="wv8p", tag="wv8p")
                nc.sync.dma_start(wv8p[:, :, :], wv8p_d[:, :, :])
            # secondary constants ride the ACT DGE queue: HWDGE descriptor
            # generation (~625ns/DMA) is serialized per queue, and these
            # would otherwise delay the first x/weight arrivals on SP
            bqk_sb = wpool.tile([HD, 2 * H], F32, name="bqk", tag="bqk")
            nc.sync.dma_start(bqk_sb[:], bqk_d[:, :])
            wvp = wpool.tile([P, 3 * D], BF16, name="wvp", tag="wvp")
            nc.sync.dma_start(wvp[:], wvp_d[:, :])
            wvb_sb = [wvp[:, D * t:D * t + D] for t in range(3)]
            vbb_sb = wpool.tile([P, D], F32, name="vbb", tag="vbb")
            nc.sync.dma_start(vbb_sb[:], vbb_d[:, :])
            msk_sb = wpool.tile([P, P], BF16, name="msk", tag="msk")
            nc.sync.dma_start(msk_sb[:], msk_d[:, :])
            m8_sb = wpool.tile([P, 2, 768], F8, name="m8", tag="m8")
            nc.sync.dma_start(m8_sb[:, :, :], m8_d[:, :, :])
            wop = wpool.tile([97, H * D], BF16, name="wop", tag="wop")
            nc.sync.dma_start(wop[:], wop_d[:, :])
            sh8_sb = wpool.tile([P, 1], F32, name="sh8", tag="sh8")
            nc.vector.memset(sh8_sb[:], B_FP8)
            one97 = wpool.tile([1, 97], F32, name="one97", tag="one97")
            nc.vector.memset(one97[:], 1.0)
            # views: composite Pool mask [.,.,0:256], DVE additive mask rest
            wq_sb = [wqp[:, D * t:D * t + D] for t in range(3)]
            wk_sb = [wkp[:, D * t:D * t + D] for t in range(3)]
            wo_sb = [wop[:, D * h:D * h + D] for h in range(H)]

            qt_sb, kt_sb = [], []
            for h in range(H):
                qt = qkt_pool.tile([HD, S], BF16, name=f"qt{h}", tag=f"qt{h}")
                qt_sb.append(qt)
                kt = qkt_pool.tile([HD, S], BF16, name=f"kt{h}", tag=f"kt{h}")
                kt_sb.append(kt)

            # V tiles: per head 128-col block, col 0 = ones (denominator),
            # cols 1..96 = dims, 97..127 junk pads (land in unread acc rows)
            vall8 = vpool.tile([P, KTN, 4 * P], F8, name="vall8", tag="vall8")
            ones8 = bass.AP(vall8.tensor, vall8[:, 0, 0:1].offset,
                            [list(vall8[:, 0, 0:1].ap[0]), [4 * P, KTN], [P, H]])
            nc.vector.memset(ones8, 1.0)
            vallb = vpool.tile([P, 4, 97 * H], BF16, name="vallb", tag="vallb")
            onesb = bass.AP(vallb.tensor, vallb[:, 0, 0:1].offset,
                            [list(vallb[:, 0, 0:1].ap[0]), [97 * H, 4], [97, H]])
            nc.vector.memset(onesb, 1.0)

            def proj_units(ci):
                """Projection work units for chunk ci: 8 Q/K + 4 V' closures."""
                units = []

                def qk_unit(w_sb, boff, dst, h):
                    def emit():
                        ps = fpool.tile([HD, CH], F32, name="projps", tag="f")
                        for t in range(3):
                            nc.tensor.matmul(
                                ps[:],
                                w_sb[t][:, HD * h:HD * h + HD],
                                xt_sb[t][:, CH * ci:CH * ci + CH],
                                start=(t == 0), stop=(t == 2))
                        nc.vector.tensor_scalar_add(
                            dst[h][:, CH * ci:CH * ci + CH], ps[:],
                            bqk_sb[:, boff + h:boff + h + 1])
                    return emit

                def v_unit(st):
                    def emit():
                        vbs = bass.AP(vbb_sb.tensor, vbb_sb.offset,
                                      [list(vbb_sb[:, 0:1].ap[0]), [HD, H], [1, HD]])
                        ps = fpool.tile([P, D], F32, name="vps", tag="f")
                        if "vdr" in VARIANT:
                            # fp8 DoubleRow projection: faster, but the fp8
                            # x/Wv noise roughly doubles the output l2 error
                            nc.tensor.matmul(ps[:],
                                             x8p[:, 0:2, P * st:P * st + P],
                                             wv8p[:, 0:2, :], start=True, stop=False,
                                             perf_mode=DR, skip_group_check=True)
                            nc.tensor.matmul(ps[:],
                                             x8p[:, 2, P * st:P * st + P],
                                             wv8p[:, 2, :], start=False, stop=True,
                                             skip_group_check=True)
                        else:
                            for t in range(3):
                                nc.tensor.matmul(ps[:],
                                                 xt_sb[t][:, P * st:P * st + P],
                                                 wvb_sb[t][:],
                                                 start=(t == 0), stop=(t == 2))
                        src = bass.AP(ps.tensor, ps.offset,
                                      [list(ps[:, 0:1].ap[0]), [HD, H], [1, HD]])
                        vdst = vall8[:, st, 0:1]
                        dst = bass.AP(vdst.tensor, vdst.offset + 1,
                                      [list(vdst.ap[0]), [P, H], [1, HD]])
                        nc.vector.tensor_tensor(dst, src, vbs, op=add)
                        if st < 4:
                            bdst0 = vallb[:, st, 0:1]
                            bdst = bass.AP(bdst0.tensor, bdst0.offset + 1,
                                           [list(bdst0.ap[0]), [97, H], [1, HD]])
                            nc.vector.tensor_tensor(bdst, src, vbs, op=add)
                    return emit

                for h in range(H):
                    units.append(qk_unit(wq_sb, 0, qt_sb, h))
                    units.append(qk_unit(wk_sb, H, kt_sb, h))
                for st in range(4 * ci, 4 * ci + 4):
                    units.append(v_unit(st))
                return units

            def outproj_units(ci, on_tiles, last=False):
                fs = onpool.tile([P, 4, D], F32, name="fs", tag="fs", bufs=2)
                units = []

                def o_unit(sj):
                    def emit():
                        fo = fpool.tile([P, D], F32, name="fo", tag="f")
                        for h in range(H):
                            nc.tensor.matmul(fo[:], on_tiles[h][:, P * sj:P * sj + P],
                                             wo_sb[h][:], start=(h == 0), stop=(h == 3))
                        if "outdve" in VARIANT:
                            nc.vector.tensor_copy(fs[:, sj, :], fo[:])
                        else:
                            nc.scalar.copy(fs[:, sj, :], fo[:])
                    return emit

                def dma_unit():
                    dst = bass.AP(out_d.tensor, 512 * ci * D,
                                  [[D, P], [P * D, 4], [1, D]])
                    nc.sync.dma_start(dst, fs[:, :, :])

                def sj_dma_unit(sj):
                    def emit():
                        dst = bass.AP(out_d.tensor, (512 * ci + P * sj) * D,
                                      [[D, P], [1, D]])
                        nc.sync.dma_start(dst, fs[:, sj, :])
                    return emit

                if last:
                    # final chunk: per-tile DMAs so the store overlaps the
                    # remaining output-projection compute in the tail
                    for sj in range(4):
                        units.append(o_unit(sj))
                        units.append(sj_dma_unit(sj))
                else:
                    for sj in range(4):
                        units.append(o_unit(sj))
                    units.append(dma_unit)
                return units

            def attend_chunk(ci, filler):
                """Attention for chunk ci; between pair iterations, emit
                independent filler units to keep PE fed during exp waits."""
                on_tiles = []
                nkt = 4 * (ci + 1)
                npairs = H * (nkt // 2)
                fill_every = max(1, npairs // max(1, len(filler)))
                fi = 0
                pair_no = 0
                fp8 = ci >= 1

                def emit_pair_a(h, pr):
                    """QK matmuls + exp/mask for one pair; returns the PV
                    closure so both heads' exps are issued before either
                    PV — PE executes in order, so this lets head h's exp
                    overlap head h+1's QK instead of stalling on PV(h)."""
                    qk = qkpool.tile([P, 2, CH], F32, name="qk", tag="qk")
                    kt0 = 2 * pr
                    rt0 = P * kt0 - CH * ci
                    sc = max(rt0, 0)
                    diag = rt0 >= 0
                    for j in range(2):
                        nc.tensor.matmul(
                            qk[:, j, sc:CH],
                            kt_sb[h][:, P * (kt0 + j):P * (kt0 + j) + P],
                            qt_sb[h][:, CH * ci + sc:CH * (ci + 1)],
                            start=True, stop=True)
                    if not fp8:
                        # chunk 0: bf16 path, ACT exp + DVE strided mask
                        pt = ppool.tile([P, 2, CH], BF16, name="pt", tag="pt")
                        nc.scalar.activation(pt[:, :, sc:CH], qk[:, :, sc:CH],
                                             Exp, scale=float(SCALE))
                        if diag:
                            base = pt[:, 0, 0:1]
                            diag_view = bass.AP(
                                base.tensor, base.offset + rt0,
                                [[2 * CH, P], [CH + P, 2], [1, P]])
                            mbc = msk_sb[:].unsqueeze(1).broadcast_to([P, 2, P])
                            nc.vector.tensor_tensor(diag_view, diag_view,
                                                    mbc, op=mult)

                        def do_pv(acc):
                            for j in range(2):
                                kt = kt0 + j
                                scol = max(P * kt - CH * ci, 0)
                                nc.tensor.matmul(
                                    acc[0:97, scol:CH],
                                    vallb[:, kt, 97 * h:97 * h + 97],
                                    pt[:, j, scol:CH],
                                    start=(kt == 0), stop=(kt == nkt - 1),
                                    skip_group_check=True)
                        return do_pv

                    use_dve = ((diag and (ci, h) not in ACT_DIAG)
                               or (not diag and (ci, h, pr) in DVE_OFF))
                    if use_dve:
                        # u8 piecewise exp, causal mask fused additively
                        pt = ppool.tile([P, 2, CH], U8, name="pt", tag="pt")
                        if diag:
                            nc.vector.scalar_tensor_tensor(
                                pt[:, :, sc:CH], qk[:, :, sc:CH], A_U8,
                                m8_sb[:, :, 256:256 + CH - sc],
                                op0=mult, op1=add)
                        else:
                            nc.vector.tensor_scalar(
                                pt[:, :, :], qk[:, :, :], A_U8, ADD32,
                                op0=mult, op1=add)
                        pt_mv = pt[:, :, sc:CH].bitcast(F8)
                    else:
                        pt = ppool.tile([P, 2, CH], F8, name="pt", tag="pt")
                        nc.scalar.activation(pt[:, :, sc:CH], qk[:, :, sc:CH],
                                             Exp, scale=float(SCALE),
                                             bias=sh8_sb[:])
                        if diag:
                            mask_eng = (nc.gpsimd if "poolmask" in VARIANT
                                        else nc.vector)
                            mask_eng.tensor_tensor(
                                pt[:, :, rt0:rt0 + 256],
                                pt[:, :, rt0:rt0 + 256],
                                m8_sb[:, :, 0:256], op=mult)
                        pt_mv = pt[:, :, sc:CH]

                    def do_pv(acc):
                        nc.tensor.matmul(
                            acc[:, sc:CH],
                            vall8[:, kt0:kt0 + 2, P * h:P * h + P],
                            pt_mv,
                            start=(pr == 0), stop=(pr == nkt // 2 - 1),
                            skip_group_check=True,
                            perf_mode=DR)
                    return do_pv

                for hh in range(0, H, 2):
                    acc0 = accpool.tile([P, CH], F32, name="acc", tag="acc")
                    acc1 = accpool.tile([P, CH], F32, name="acc", tag="acc")
                    for pr in range(nkt // 2):
                        pv0 = emit_pair_a(hh, pr)
                        pv1 = emit_pair_a(hh + 1, pr)
                        pv0(acc0)
                        pv1(acc1)
                        pair_no += 2
                        if pair_no % fill_every < 2 and fi < len(filler):
                            filler[fi]()
                            fi += 1
                    rec2 = rpool.tile([1, 2 * CH], F32, name="rec", tag="rec")
                    nc.vector.reciprocal_approx_fast(
                        out=rec2[0:1, 0:CH], in_=acc0[0:1, :])
                    nc.vector.reciprocal_approx_fast(
                        out=rec2[0:1, CH:2 * CH], in_=acc1[0:1, :])
                    if "pebc" in VARIANT:
                        # broadcast 1/denom across partitions on the PE:
                        # ones[1,97]^T @ rec2[1,:] -> PSUM [97, CH]
                        for kk, a in ((0, acc0), (1, acc1)):
                            rbp = fpool.tile([97, CH], F32, name="rbp", tag="f")
                            nc.tensor.matmul(
                                rbp[:], one97[:],
                                rec2[0:1, CH * kk:CH * kk + CH],
                                start=True, stop=True)
                            on = onpool.tile([97, CH], BF16, name=f"on{hh + kk}",
                                             tag=f"on{hh + kk}")
                            nc.vector.tensor_tensor(
                                on[:], a[0:97, :], rbp[:], op=mult)
                            on_tiles.append(on)
                    else:
                        rb2 = rpool.tile([97, 2 * CH], F32, name="rb", tag="rb")
                        if "fakebc" in VARIANT:
                            # timing ablation: DVE memset instead of Pool
                            # broadcast (WRONG results)
                            nc.vector.memset(rb2[:], 0.01)
                        else:
                            nc.gpsimd.partition_broadcast(rb2[:], rec2[:],
                                                          channels=97)
                        for kk, a in ((0, acc0), (1, acc1)):
                            on = onpool.tile([97, CH], BF16, name=f"on{hh + kk}",
                                             tag=f"on{hh + kk}")
                            nc.vector.tensor_tensor(
                                on[:], a[0:97, :], rb2[:, CH * kk:CH * kk + CH],
                                op=mult)
                            on_tiles.append(on)
                # unplaced fillers burst at the chunk boundary; this is
                # load-bearing for the next chunk's projections (attend(ci+1)
                # reads their tiles in program order — leaking them forward
                # would read stale SBUF)
                while fi < len(filler):
                    filler[fi]()
                    fi += 1
                return on_tiles

            # software pipeline across chunks: during attention of chunk ci,
            # emit chunk ci+1's projections and chunk ci-1's output
            # projection.  NOTE the chunk order is forced: attend(ci) needs
            # the K/V prefix from proj(0..ci), so attention must run 0,1,2,3.
            for u in proj_units(0):
                u()
            pending_out = []
            for ci in range(NCH):
                filler = list(pending_out)
                if ci + 1 < NCH:
                    filler += proj_units(ci + 1)
                on_tiles = attend_chunk(ci, filler)
                pending_out = outproj_units(ci, on_tiles, last=(ci == NCH - 1))
            for u in pending_out:
                u()

        for pool in (fpool, accpool, qkpool, rpool, onpool, ppool, vpool,
                     qkt_pool, xpool, wpool):
            pool.release()

    nc.finalize()
    return nc


_NC_CACHE = None


def get_nc():
    global _NC_CACHE
    if _NC_CACHE is None:
        _NC_CACHE = build_nc()
    return _NC_CACHE


def host_prep(x, Wq, bq, Wk, bk, Wv, bv, Wo, bo):
    """Build per-core input maps (layout prep only; all FLOPs run on device)."""
    BF = ml_dtypes.bfloat16
    F8H = ml_dtypes.float8_e4m3
    x = np.ascontiguousarray(np.asarray(x, dtype=np.float32))
    Wq = np.ascontiguousarray(np.asarray(Wq, dtype=np.float32))
    Wk = np.ascontiguousarray(np.asarray(Wk, dtype=np.float32))
    Wv = np.ascontiguousarray(np.asarray(Wv, dtype=np.float32))
    Wo = np.ascontiguousarray(np.asarray(Wo, dtype=np.float32))
    bq = np.asarray(bq, dtype=np.float32)
    bk = np.asarray(bk, dtype=np.float32)
    bv = np.asarray(bv, dtype=np.float32)
    bo = np.asarray(bo, dtype=np.float32)

    wqp = np.hstack([Wq[0:P], Wq[P:2 * P], Wq[2 * P:3 * P]]).astype(BF)
    wkp = np.hstack([Wk[0:P], Wk[P:2 * P], Wk[2 * P:3 * P]]).astype(BF)
    wvp = np.hstack([Wv[0:P], Wv[P:2 * P], Wv[2 * P:3 * P]]).astype(BF)
    wv8p = np.stack([Wv[0:P], Wv[P:2 * P], Wv[2 * P:3 * P]], axis=1).astype(F8H)
    vbb = np.ascontiguousarray(np.broadcast_to(bv.reshape(1, D), (P, D)))

    # Wo with per-head 97-row blocks; row 0 picks up bo via on[0] == 1.0
    wop = np.zeros((97, H * D), np.float32)
    for h in range(H):
        wop[0, D * h:D * h + D] = bo / H
        wop[1:97, D * h:D * h + D] = Wo[HD * h:HD * h + HD, :]

    jj = np.arange(P)[None, :]
    pp = np.arange(P)[:, None]
    msk = (jj >= pp).astype(BF)
    tri = (jj >= pp).astype(np.float32)
    # m8[:, :, 0:256]: Pool composite {0,1}: [tri|ones] slab0, [zero|tri] slab1
    # m8[:, :, 256:768]: DVE additive mask: +32 valid, -448 masked
    m8 = np.zeros((P, 2, 768), np.float32)
    m8[:, 0, 0:P] = tri
    m8[:, 0, P:256] = 1.0
    m8[:, 1, P:256] = tri
    ja = np.arange(CH)[None, :]
    m8[:, 0, 256:768] = np.where(ja >= pp, ADD32, -448.0)
    m8[:, 1, 256:768] = np.where(ja >= P + pp, ADD32, -448.0)
    m8 = m8.astype(F8H)

    bqk = np.hstack([bq.reshape(H, HD).T, bk.reshape(H, HD).T])
    bqk = np.ascontiguousarray(bqk)

    common = dict(wqp=wqp, wkp=wkp, wvp=wvp, wv8p=wv8p, wop=wop.astype(BF),
                  bqk=bqk, vbb=vbb, msk=msk, m8=m8)
    maps = []
    for b in range(x.shape[0]):
        xt = np.ascontiguousarray(x[b].T)  # [D, S]
        x8p = np.stack([xt[0:P], xt[P:2 * P], xt[2 * P:3 * P]], axis=1).astype(F8H)
        maps.append(dict(xt=xt.astype(BF), x8p=x8p, **common))
    return maps


def kernel(**inputs):
    in_maps = host_prep(**inputs)
    nc = get_nc()
    res = run_bass_kernel_spmd(nc, in_maps, core_ids=list(range(N_CORES)))
    return np.stack([res.results[b]["out"] for b in range(N_CORES)], axis=0)
